# revision 10
# baseline (speedup 1.0000x reference)
"""CBTree bottom-up fold kernel for 8 trn2 NeuronCores.

Problem: complete 4-ary tree, 9 levels, 87381 nodes in BFS order, d=256.
  leaves (level 8): h = vectors[21845:]
  internal node:    h = tanh(sum_i W_i @ h_child_i + vectors[node])
  where W_i = lc[i]*Wl + rc[i]*Wr,  lc=[1,2/3,1/3,0], rc=[0,1/3,2/3,1].

Strategy (data-parallel over sibling groups):
  - Shard every level contiguously over 8 cores. Children of a core's
    parents are exactly the core's own previous-level outputs, so
    levels 7..2 run with zero communication (level-2 node j's children
    [4j,4j+4) lie inside core j//2's level-3 slice [8r,8r+8)).
  - One tiny AllGather of the level-2 states (16 nodes), then every
    core redundantly folds levels 1..0 and writes the root.
  - On chip h lives transposed ([d, nodes], d split into two 128-row
    partition halves) so the tensor engine contracts over d; the host
    hands each core its slices already in this layout.
  - The per-node bias vector is DVE-preloaded into the PSUM bank and
    the 8 child matmuls accumulate onto it (start=False), so each
    parent costs 16 PE cycles instead of 18.
  - fp16 everywhere on chip (fp32 PSUM accumulation): ~3.4e-3
    scale-relative error vs the fp32 reference. fp8/bf16 blow the
    2e-2 budget (error amplifies ~20x through the 8-level fold).
  - The root would be an N=1 matmul (invalid ISA), so the last level
    computes 4 replicated copies of the root; the output DMA reads
    copy 0 with a strided AP straight into the [1,256] fp16 out
    tensor (host upcasts to f32).
"""

import numpy as np

F16 = None  # set on first _lazy_imports()

_BASS = {}


def _lazy_imports():
    global bass, bacc, mybir, tile, run_bass_kernel_spmd, F16, F32
    import concourse.bass as bass
    import concourse.mybir as mybir
    from concourse import bacc
    import concourse.tile as tile
    from concourse.bass_utils import run_bass_kernel_spmd
    F16 = mybir.dt.float16
    F32 = mybir.dt.float32

N_CORES = 8
D = 256
B = 4
L = 9
SIZES = [B**l for l in range(L)]            # [1,4,16,64,256,1024,4096,16384,65536]
OFFSETS = np.concatenate([[0], np.cumsum(SIZES)])
N_LEAF_CORE = SIZES[8] // N_CORES           # 8192
LOC_LEVELS = [7, 6, 5, 4, 3, 2]
LOC_PAR = {l: SIZES[l] // N_CORES for l in LOC_LEVELS}  # 2048,512,128,32,8,2
N_VECS_LOC = sum(LOC_PAR.values())          # 2730
N_VECS_TAIL = 4 + SIZES[1]                  # 4x replicated root + 4 L1 nodes
# level-7 compute chunk sizes (parents); leaf DMAs are 4x these (cols)
L7_CHUNKS = [128, 384, 512, 512, 512]


def _build_nc(mode="fp16", l7_chunks=None, chunks=None):
    if l7_chunks is None:
        l7_chunks = L7_CHUNKS
    if chunks is None:
        chunks = {}
    key = ("nc", mode, tuple(l7_chunks), tuple(sorted(chunks.items())))
    if key in _BASS:
        return _BASS[key]
    assert mode == "fp16"
    nc = bacc.Bacc(num_devices=N_CORES)

    # all h/vec tensors arrive transposed: [256 = 2x128 d-rows, n nodes]
    leavesT = nc.declare_dram_parameter("leavesT", [D, N_LEAF_CORE], F16, isOutput=False)
    vecs_locT = nc.declare_dram_parameter("vecs_locT", [D, N_VECS_LOC], F16, isOutput=False)
    vecs_tailT = nc.declare_dram_parameter("vecs_tailT", [D, N_VECS_TAIL], F16, isOutput=False)
    wmat = nc.declare_dram_parameter("wmat", [128, 17 * 128], F16, isOutput=False)
    out = nc.declare_dram_parameter("out", [1, D], F16, isOutput=True)

    with tile.TileContext(nc) as tc:
        with (
            tc.tile_pool(name="const", bufs=1) as const_pool,
            tc.tile_pool(name="hbuf", bufs=1) as hbuf,
            tc.tile_pool(name="vecp", bufs=1) as vec_pool,
            tc.tile_pool(name="pmm", bufs=6, space="PSUM") as psum_mm,
            tc.tile_pool(name="dram", bufs=1, space="DRAM") as dram_pool,
        ):
            # weight blocks: mh=0 blocks (i,kh)=0..7, identity at 8, mh=1 at 9..16
            wsb = const_pool.tile([128, 17 * 128], F16, name="wsb")
            nc.sync.dma_start(wsb[:], wmat[:])

            # activation-table warm: tiny tanh on a zeroed tile so the
            # 1283ns table load hides under the initial DMA shadow
            warm = const_pool.tile([128, 4], F32, name="warm")
            nc.gpsimd.memset(warm[:], 0.0)
            nc.scalar.activation(warm[:1, :4], warm[:1, :4],
                                 mybir.ActivationFunctionType.Tanh)

            # persistent transposed h states, one tile per (level, d-half)
            def h_tiles(name, n):
                return [hbuf.tile([128, max(n, 1)], F16, name=f"{name}_{kh}",
                                  tag=f"{name}_{kh}") for kh in (0, 1)]

            hT8 = h_tiles("hT8", N_LEAF_CORE)
            hT = {7: h_tiles("hT7", 2048), 6: h_tiles("hT6", 512),
                  5: h_tiles("hT5", 128), 4: h_tiles("hT4", 32),
                  3: h_tiles("hT3", 8)}
            # level-2 state packed [128, kh, n] so the AG bounce is 1 DMA
            NL2 = LOC_PAR[2]                                # 2
            t2p = hbuf.tile([128, 2, NL2], F16, name="hT2p", tag="hT2p")
            hT[2] = [t2p[:, 0, :], t2p[:, 1, :]]
            # gathered level-2 states [128, kh, 16]
            h2ap = hbuf.tile([128, 2, SIZES[2]], F16, name="h2allp", tag="h2allp")
            # tail tiles
            t1p = hbuf.tile([128, 2, SIZES[1]], F16, name="hT1p", tag="hT1p")
            t0p = hbuf.tile([128, 2, 4], F16, name="hT0p", tag="hT0p")

            vloc = vec_pool.tile([128, 2, N_VECS_LOC], F16, name="vloc", tag="vloc")
            vtail = vec_pool.tile([128, 2, N_VECS_TAIL], F16, name="vtail", tag="vtail")

            def vec_dma(tile_ap, dram_t, col0, n):
                nc.scalar.dma_start(
                    tile_ap[:, :, col0:col0 + n],
                    dram_t[:, col0:col0 + n].rearrange("(mh k) n -> k mh n", mh=2))

            def leaf_dma(col0, n):
                for kh in (0, 1):
                    nc.sync.dma_start(
                        hT8[kh][:, col0:col0 + n],
                        leavesT[kh * 128:(kh + 1) * 128, col0:col0 + n])

            # ---- shared level routine (bias preloaded into PSUM) ----
            def do_level(child, n_par, vec_tile, vec_col0, hT_out,
                         chunk_prologue=None, chunk=512, rview_override=None):
                rview = rview_override or [
                    child[kh][:, :4 * n_par].rearrange(
                        "k (p four) -> k p four", four=4)
                    for kh in (0, 1)]
                for c0 in range(0, n_par, chunk):
                    if chunk_prologue is not None:
                        chunk_prologue(c0)
                    N = min(chunk, n_par - c0)
                    for mh in (0, 1):
                        ps = psum_mm.tile([128, 512], F32, name="ps_mm", tag="mm")
                        for i in range(4):
                            for kh in (0, 1):
                                blk = (9 if mh else 0) + i * 2 + kh
                                w = wsb[:, blk * 128:(blk + 1) * 128]
                                rhs = rview[kh][:, c0:c0 + N, i]
                                nc.tensor.matmul(ps[:, :N], w, rhs,
                                                 start=(i == 0 and kh == 0),
                                                 stop=False)
                        nc.tensor.matmul(
                            ps[:, :N], wsb[:, 8 * 128:9 * 128],
                            vec_tile[:, mh, vec_col0 + c0: vec_col0 + c0 + N],
                            start=False, stop=True)
                        nc.scalar.activation(hT_out[mh][:, c0:c0 + N], ps[:, :N],
                                             mybir.ActivationFunctionType.Tanh)

            # ---- level 7: interleave leaf/vec DMA stream with compute ----
            bounds = np.concatenate([[0], np.cumsum(l7_chunks)])
            assert bounds[-1] == LOC_PAR[7]

            def l7_prologue(c0):
                ci = int(np.searchsorted(bounds, c0))
                # stream chunk ci+1's data while chunk ci computes
                nxt = ci + 1
                if nxt < len(l7_chunks):
                    p0, p1 = int(bounds[nxt]), int(bounds[nxt + 1])
                    leaf_dma(4 * p0, 4 * (p1 - p0))
                    vec_dma(vloc, vecs_locT, p0, p1 - p0)
                elif nxt == len(l7_chunks):
                    # rest of the bias vectors (levels 6..2) + tail vecs
                    vec_dma(vloc, vecs_locT, LOC_PAR[7], N_VECS_LOC - LOC_PAR[7])
                    nc.scalar.dma_start(
                        vtail[:],
                        vecs_tailT[:].rearrange("(mh k) n -> k mh n", mh=2))

            # prime the pipeline: chunk 0's leaves+vecs
            leaf_dma(0, 4 * l7_chunks[0])
            vec_dma(vloc, vecs_locT, 0, l7_chunks[0])

            ci = [0]

            def l7_chunked(c0):
                l7_prologue(c0)

            # run level 7 with the variable chunk list
            rview7 = [hT8[kh][:].rearrange("k (p four) -> k p four", four=4)
                      for kh in (0, 1)]
            for k, n in enumerate(l7_chunks):
                c0 = int(bounds[k])
                l7_prologue(c0)
                for mh in (0, 1):
                    ps = psum_mm.tile([128, 512], F32, name="ps_mm", tag="mm")
                    for i in range(4):
                        for kh in (0, 1):
                            blk = (9 if mh else 0) + i * 2 + kh
                            w = wsb[:, blk * 128:(blk + 1) * 128]
                            rhs = rview7[kh][:, c0:c0 + n, i]
                            nc.tensor.matmul(ps[:, :n], w, rhs,
                                             start=(i == 0 and kh == 0),
                                             stop=False)
                    nc.tensor.matmul(ps[:, :n], wsb[:, 8 * 128:9 * 128],
                                     vloc[:, mh, c0:c0 + n],
                                     start=False, stop=True)
                    nc.scalar.activation(hT[7][mh][:, c0:c0 + n], ps[:, :n],
                                         mybir.ActivationFunctionType.Tanh)

            # ---- local levels 6..2 ----
            col0 = LOC_PAR[7]
            child = hT[7]
            for l in [6, 5, 4, 3, 2]:
                do_level(child, LOC_PAR[l], vloc, col0, hT[l],
                         chunk=chunks.get(l, 512))
                col0 += LOC_PAR[l]
                child = hT[l]

            # ---- AllGather of level-2 states (packed, 1 DMA each way) ----
            cc_in = dram_pool.tile([D, NL2], F16, name="cc_in")
            cc_out = dram_pool.tile([N_CORES * D, NL2], F16, name="cc_out")
            nc.sync.dma_start(
                cc_in[:].rearrange("(kh k) n -> k kh n", kh=2), t2p[:])
            nc.gpsimd.collective_compute(
                "AllGather", mybir.AluOpType.bypass,
                replica_groups=[list(range(N_CORES))],
                ins=[cc_in.opt()], outs=[cc_out.opt()])
            # core r's block at rows [256r, 256r+256); fetch every block's
            # two d-halves into the packed [128, kh, 16] layout
            cc_v = cc_out[:].rearrange("(r kh k) n -> kh k r n",
                                       r=N_CORES, kh=2)
            for kh in (0, 1):
                nc.sync.dma_start(
                    h2ap[:, kh, :].rearrange("k (r n) -> k r n", r=N_CORES),
                    cc_v[kh])
            h2all = [h2ap[:, 0, :], h2ap[:, 1, :]]

            # ---- replicated tail: level 1, then 4 root copies ----
            do_level(h2all, SIZES[1], vtail, 4, [t1p[:, 0, :], t1p[:, 1, :]])
            root_rv = [t1p[:, kh, 0:4].unsqueeze(1).broadcast_to([128, 4, 4])
                       for kh in (0, 1)]
            do_level(None, 4, vtail, 0, [t0p[:, 0, :], t0p[:, 1, :]],
                     rview_override=root_rv)

            # ---- write the root: strided fp16 DMA, no transpose ----
            nc.sync.dma_start(
                out[:].rearrange("o (kh k) -> k o kh", kh=2),
                t0p[:, :, 0:1].rearrange("k kh o -> k o kh"))

    nc.finalize()
    _BASS[key] = nc
    return nc


def _prep_inputs(vectors, Wl, Wr):
    vectors = np.asarray(vectors, dtype=np.float32)
    Wl = np.asarray(Wl, dtype=np.float32)
    Wr = np.asarray(Wr, dtype=np.float32)

    ind = np.arange(1, B + 1, dtype=np.float32)
    lc = (B - ind) / (B - 1)
    rc = (ind - 1) / (B - 1)
    # W_t[i] = W_i.T; block order mh0(8) | identity | mh1(8) so one DMA
    Wt = np.stack([lc[i] * Wl.T + rc[i] * Wr.T for i in range(B)])  # [4, 256k, 256m]
    W5 = Wt.reshape(4, 2, 128, 2, 128)            # [i, kh, k', mh, m']
    halves = [W5[:, :, :, mh, :].reshape(4, 2, 128, 128)
              .transpose(2, 0, 1, 3).reshape(128, 8 * 128) for mh in (0, 1)]
    wmat = np.ascontiguousarray(
        np.concatenate([halves[0], np.eye(128, dtype=np.float32), halves[1]],
                       axis=1), dtype=np.float32)

    # one transposed copy of the node array; per-core slices are views
    vecsT = np.ascontiguousarray(vectors.T)                      # [256, 87381]
    vecs_tailT = np.ascontiguousarray(
        np.concatenate([np.repeat(vecsT[:, 0:1], 4, axis=1),
                        vecsT[:, 1:5]], axis=1))
    hdt = np.float16
    in_maps = []
    for c in range(N_CORES):
        o8 = int(OFFSETS[8])
        leavesT_c = vecsT[:, o8 + c * N_LEAF_CORE: o8 + (c + 1) * N_LEAF_CORE]
        loc_parts = []
        for l in LOC_LEVELS:
            npl = LOC_PAR[l]
            o = int(OFFSETS[l])
            loc_parts.append(vecsT[:, o + c * npl: o + (c + 1) * npl])
        im = {
            "leavesT": np.ascontiguousarray(leavesT_c).astype(hdt),
            "vecs_locT": np.ascontiguousarray(
                np.concatenate(loc_parts, axis=1)).astype(hdt),
            "vecs_tailT": vecs_tailT.astype(hdt),
            "wmat": wmat.astype(hdt),
        }
        in_maps.append(im)
    return in_maps


def kernel(vectors, Wl, Wr, branching, n_levels, _mode="fp16"):
    _lazy_imports()
    assert int(branching) == B and int(n_levels) == L
    vectors = np.asarray(vectors)
    assert vectors.shape == (int(OFFSETS[L]), D), vectors.shape

    nc = _build_nc(mode=_mode)
    in_maps = _prep_inputs(vectors, Wl, Wr)
    try:
        res = run_bass_kernel_spmd(nc, in_maps, core_ids=list(range(N_CORES)),
                                   trace=False)
    except Exception:
        # transient device hiccups clear on a retry
        res = run_bass_kernel_spmd(nc, in_maps, core_ids=list(range(N_CORES)),
                                   trace=False)
    root = res.results[0]["out"]
    return np.asarray(root, dtype=np.float32).reshape(1, D)


# revision 11
# speedup vs baseline: 1.0301x; 1.0301x over previous
"""CBTree bottom-up fold kernel for 8 trn2 NeuronCores.

Problem: complete 4-ary tree, 9 levels, 87381 nodes in BFS order, d=256.
  leaves (level 8): h = vectors[21845:]
  internal node:    h = tanh(sum_i W_i @ h_child_i + vectors[node])
  where W_i = lc[i]*Wl + rc[i]*Wr,  lc=[1,2/3,1/3,0], rc=[0,1/3,2/3,1].

Strategy (data-parallel over sibling groups):
  - Shard every level contiguously over 8 cores. Children of a core's
    parents are exactly the core's own previous-level outputs, so
    levels 7..2 run with zero communication (level-2 node j's children
    [4j,4j+4) lie inside core j//2's level-3 slice [8r,8r+8)).
  - One tiny AllGather of the level-2 states (16 nodes), then every
    core redundantly folds levels 1..0 and writes the root.
  - On chip h lives transposed ([d, nodes], two 128-row partition
    halves) so the tensor engine contracts over d; the host hands each
    core its slices already in this layout.
  - All stream DMAs are issued up-front in consumption order on the SP
    queue (in-order, self-pacing); level-6 compute chunks are emitted
    interleaved into level-7's DMA-bound bubbles to keep PE busy.
  - fp16 everywhere on chip (fp32 PSUM accumulation): ~3.4e-3
    scale-relative error. fp8/bf16 blow the 2e-2 budget (error
    amplifies ~20x through the 8-level fold).
  - The root would be an N=1 matmul (invalid ISA), so the last level
    computes 4 replicated root copies; the output DMA reads copy 0
    with a strided AP straight into the [1,256] fp16 out tensor (host
    upcasts to f32).
"""

import numpy as np

F16 = None  # set on first _lazy_imports()

_BASS = {}


def _lazy_imports():
    global bass, bacc, mybir, tile, run_bass_kernel_spmd, F16, F32
    import concourse.bass as bass
    import concourse.mybir as mybir
    from concourse import bacc
    import concourse.tile as tile
    from concourse.bass_utils import run_bass_kernel_spmd
    F16 = mybir.dt.float16
    F32 = mybir.dt.float32

N_CORES = 8
D = 256
B = 4
L = 9
SIZES = [B**l for l in range(L)]            # [1,4,16,64,256,1024,4096,16384,65536]
OFFSETS = np.concatenate([[0], np.cumsum(SIZES)])
N_LEAF_CORE = SIZES[8] // N_CORES           # 8192
LOC_LEVELS = [7, 6, 5, 4, 3, 2]
LOC_PAR = {l: SIZES[l] // N_CORES for l in LOC_LEVELS}  # 2048,512,128,32,8,2
N_VECS_LOC = sum(LOC_PAR.values())          # 2730
N_VECS_TAIL = 4 + SIZES[1]                  # 4x replicated root + 4 L1 nodes
L7_CHUNKS = [128, 384, 512, 512, 512]
L6_CHUNKS = [128, 128, 128, 128]


def _build_nc(mode="fp16"):
    key = ("nc", mode)
    if key in _BASS:
        return _BASS[key]
    assert mode == "fp16"
    nc = bacc.Bacc(num_devices=N_CORES)

    leavesT = nc.declare_dram_parameter("leavesT", [D, N_LEAF_CORE], F16, isOutput=False)
    vecs_locT = nc.declare_dram_parameter("vecs_locT", [D, N_VECS_LOC], F16, isOutput=False)
    vecs_tailT = nc.declare_dram_parameter("vecs_tailT", [D, N_VECS_TAIL], F16, isOutput=False)
    wmat = nc.declare_dram_parameter("wmat", [128, 17 * 128], F16, isOutput=False)
    out = nc.declare_dram_parameter("out", [1, D], F16, isOutput=True)

    with tile.TileContext(nc) as tc:
        with (
            tc.tile_pool(name="const", bufs=1) as const_pool,
            tc.tile_pool(name="hbuf", bufs=1) as hbuf,
            tc.tile_pool(name="vecp", bufs=1) as vec_pool,
            tc.tile_pool(name="pmm", bufs=6, space="PSUM") as psum_mm,
            tc.tile_pool(name="dram", bufs=1, space="DRAM") as dram_pool,
        ):
            # weight blocks: mh=0 blocks (i,kh)=0..7, identity at 8, mh=1 at 9..16
            wsb = const_pool.tile([128, 17 * 128], F16, name="wsb")

            # activation-table warm on a zeroed tile
            warm = const_pool.tile([128, 4], F32, name="warm")
            nc.gpsimd.memset(warm[:], 0.0)
            nc.scalar.activation(warm[:1, :4], warm[:1, :4],
                                 mybir.ActivationFunctionType.Tanh)

            def h_tiles(name, n):
                return [hbuf.tile([128, max(n, 1)], F16, name=f"{name}_{kh}",
                                  tag=f"{name}_{kh}") for kh in (0, 1)]

            hT8 = h_tiles("hT8", N_LEAF_CORE)
            hT = {7: h_tiles("hT7", 2048), 6: h_tiles("hT6", 512),
                  5: h_tiles("hT5", 128), 4: h_tiles("hT4", 32),
                  3: h_tiles("hT3", 8)}
            NL2 = LOC_PAR[2]                                # 2
            t2p = hbuf.tile([128, 2, NL2], F16, name="hT2p", tag="hT2p")
            hT[2] = [t2p[:, 0, :], t2p[:, 1, :]]
            # gathered level-2 states, columns ordered (r, kh, n) so the
            # unbounce is ONE 3D DMA; level-1 rhs uses stride-8 column APs
            h2g = hbuf.tile([128, 2 * SIZES[2]], F16, name="h2g", tag="h2g")
            t1p = hbuf.tile([128, 2, SIZES[1]], F16, name="hT1p", tag="hT1p")
            t0p = hbuf.tile([128, 2, 4], F16, name="hT0p", tag="hT0p")

            vloc = vec_pool.tile([128, 2, N_VECS_LOC], F16, name="vloc", tag="vloc")
            vtail = vec_pool.tile([128, 2, N_VECS_TAIL], F16, name="vtail", tag="vtail")

            def vec_dma(col0, n):
                nc.sync.dma_start(
                    vloc[:, :, col0:col0 + n],
                    vecs_locT[:, col0:col0 + n].rearrange("(mh k) n -> k mh n", mh=2))

            def leaf_dma(col0, n):
                for kh in (0, 1):
                    nc.sync.dma_start(
                        hT8[kh][:, col0:col0 + n],
                        leavesT[kh * 128:(kh + 1) * 128, col0:col0 + n])

            # ---- the full input stream, in consumption order, on SP ----
            bounds7 = np.concatenate([[0], np.cumsum(L7_CHUNKS)])
            assert bounds7[-1] == LOC_PAR[7]
            leaf_dma(0, 4 * L7_CHUNKS[0])
            vec_dma(0, L7_CHUNKS[0])
            nc.sync.dma_start(wsb[:, :9 * 128], wmat[:, :9 * 128])
            leaf_dma(4 * int(bounds7[1]), 4 * L7_CHUNKS[1])
            vec_dma(int(bounds7[1]), L7_CHUNKS[1])
            nc.sync.dma_start(wsb[:, 9 * 128:], wmat[:, 9 * 128:])
            leaf_dma(4 * int(bounds7[2]), 4 * L7_CHUNKS[2])
            vec_dma(int(bounds7[2]), L7_CHUNKS[2])
            # levels 6..2 bias vectors + tail vecs, needed from ~mid-L7
            vec_dma(LOC_PAR[7], N_VECS_LOC - LOC_PAR[7])
            nc.sync.dma_start(
                vtail[:],
                vecs_tailT[:].rearrange("(mh k) n -> k mh n", mh=2))
            leaf_dma(4 * int(bounds7[3]), 4 * L7_CHUNKS[3])
            vec_dma(int(bounds7[3]), L7_CHUNKS[3])
            leaf_dma(4 * int(bounds7[4]), 4 * L7_CHUNKS[4])
            vec_dma(int(bounds7[4]), L7_CHUNKS[4])

            # ---- compute: one (level, chunk) psum-group pair ----
            def do_chunk(rview, c0, N, vec_tile, vec_col0, hT_out):
                for mh in (0, 1):
                    ps = psum_mm.tile([128, 512], F32, name="ps_mm", tag="mm")
                    for kh in (0, 1):
                        for i in range(4):
                            blk = (9 if mh else 0) + i * 2 + kh
                            w = wsb[:, blk * 128:(blk + 1) * 128]
                            rhs = rview[kh][:, c0:c0 + N, i]
                            nc.tensor.matmul(ps[:, :N], w, rhs,
                                             start=(i == 0 and kh == 0),
                                             stop=False)
                    nc.tensor.matmul(
                        ps[:, :N], wsb[:, 8 * 128:9 * 128],
                        vec_tile[:, mh, vec_col0 + c0: vec_col0 + c0 + N],
                        start=False, stop=True)
                    nc.scalar.activation(hT_out[mh][:, c0:c0 + N], ps[:, :N],
                                         mybir.ActivationFunctionType.Tanh)

            def rv(child, n_par):
                return [child[kh][:, :4 * n_par].rearrange(
                    "k (p four) -> k p four", four=4) for kh in (0, 1)]

            rview7 = rv(hT8, LOC_PAR[7])
            rview6 = rv(hT[7], LOC_PAR[6])
            # interleave: L6 chunk j after the L7 chunks that produce its input
            bounds6 = np.concatenate([[0], np.cumsum(L6_CHUNKS)])
            vcol = {7: 0, 6: LOC_PAR[7]}
            plan = [(7, 0), (7, 1), (6, 0), (7, 2), (6, 1), (7, 3), (6, 2),
                    (7, 4), (6, 3)]
            for lvl, k in plan:
                if lvl == 7:
                    c0, n = int(bounds7[k]), L7_CHUNKS[k]
                    do_chunk(rview7, c0, n, vloc, 0, hT[7])
                else:
                    c0, n = int(bounds6[k]), L6_CHUNKS[k]
                    do_chunk(rview6, c0, n, vloc, vcol[6], hT[6])

            # ---- levels 5..2 ----
            col0 = LOC_PAR[7] + LOC_PAR[6]
            child = hT[6]
            for l in (5, 4, 3, 2):
                do_chunk(rv(child, LOC_PAR[l]), 0, LOC_PAR[l], vloc, col0, hT[l])
                col0 += LOC_PAR[l]
                child = hT[l]

            # ---- AllGather of level-2 states ----
            cc_in = dram_pool.tile([D, NL2], F16, name="cc_in")
            cc_out = dram_pool.tile([N_CORES * D, NL2], F16, name="cc_out")
            nc.sync.dma_start(
                cc_in[:].rearrange("(kh k) n -> k kh n", kh=2), t2p[:])
            nc.gpsimd.collective_compute(
                "AllGather", mybir.AluOpType.bypass,
                replica_groups=[list(range(N_CORES))],
                ins=[cc_in.opt()], outs=[cc_out.opt()])
            # gathered rows are (q=(r,kh), k); one 3D DMA into columns (q, n)
            nc.sync.dma_start(
                h2g[:].rearrange("k (q n) -> k q n", n=NL2),
                cc_out[:].rearrange("(q k) n -> k q n", k=128))

            # ---- tail: level 1 (4 parents), then 4 root copies ----
            # L2 node m=4j+i lives at column 8j + 4*(i//2) + 2*kh + i%2
            h2r = h2g[:].rearrange("k (j e) -> k j e", e=8)
            rview1 = None  # custom per-(i,kh) columns

            def do_tail(rhs_fn, N, vec_col0, out_tile):
                for mh in (0, 1):
                    ps = psum_mm.tile([128, 512], F32, name="ps_mm", tag="mm")
                    for kh in (0, 1):
                        for i in range(4):
                            blk = (9 if mh else 0) + i * 2 + kh
                            w = wsb[:, blk * 128:(blk + 1) * 128]
                            nc.tensor.matmul(ps[:, :N], w, rhs_fn(i, kh),
                                             start=(i == 0 and kh == 0),
                                             stop=False)
                    nc.tensor.matmul(
                        ps[:, :N], wsb[:, 8 * 128:9 * 128],
                        vtail[:, mh, vec_col0:vec_col0 + N],
                        start=False, stop=True)
                    nc.scalar.activation(out_tile[:, mh, :N], ps[:, :N],
                                         mybir.ActivationFunctionType.Tanh)

            do_tail(lambda i, kh: h2r[:, :, 4 * (i // 2) + 2 * kh + (i % 2)],
                    SIZES[1], 4, t1p)
            do_tail(lambda i, kh: t1p[:, kh, i].unsqueeze(1).broadcast_to([128, 4]),
                    4, 0, t0p)

            # ---- write the root: strided fp16 DMA, no transpose ----
            nc.sync.dma_start(
                out[:].rearrange("o (kh k) -> k o kh", kh=2),
                t0p[:, :, 0:1].rearrange("k kh o -> k o kh"))

    nc.finalize()
    _BASS[key] = nc
    return nc


def _prep_inputs(vectors, Wl, Wr):
    vectors = np.asarray(vectors, dtype=np.float32)
    Wl = np.asarray(Wl, dtype=np.float32)
    Wr = np.asarray(Wr, dtype=np.float32)

    ind = np.arange(1, B + 1, dtype=np.float32)
    lc = (B - ind) / (B - 1)
    rc = (ind - 1) / (B - 1)
    # W_t[i] = W_i.T; block order mh0(8) | identity | mh1(8)
    Wt = np.stack([lc[i] * Wl.T + rc[i] * Wr.T for i in range(B)])  # [4, 256k, 256m]
    W5 = Wt.reshape(4, 2, 128, 2, 128)            # [i, kh, k', mh, m']
    halves = [W5[:, :, :, mh, :].reshape(4, 2, 128, 128)
              .transpose(2, 0, 1, 3).reshape(128, 8 * 128) for mh in (0, 1)]
    wmat = np.ascontiguousarray(
        np.concatenate([halves[0], np.eye(128, dtype=np.float32), halves[1]],
                       axis=1), dtype=np.float32)

    vecsT = np.ascontiguousarray(vectors.T)                      # [256, 87381]
    vecs_tailT = np.ascontiguousarray(
        np.concatenate([np.repeat(vecsT[:, 0:1], 4, axis=1),
                        vecsT[:, 1:5]], axis=1))
    hdt = np.float16
    in_maps = []
    for c in range(N_CORES):
        o8 = int(OFFSETS[8])
        leavesT_c = vecsT[:, o8 + c * N_LEAF_CORE: o8 + (c + 1) * N_LEAF_CORE]
        loc_parts = []
        for l in LOC_LEVELS:
            npl = LOC_PAR[l]
            o = int(OFFSETS[l])
            loc_parts.append(vecsT[:, o + c * npl: o + (c + 1) * npl])
        im = {
            "leavesT": np.ascontiguousarray(leavesT_c).astype(hdt),
            "vecs_locT": np.ascontiguousarray(
                np.concatenate(loc_parts, axis=1)).astype(hdt),
            "vecs_tailT": vecs_tailT.astype(hdt),
            "wmat": wmat.astype(hdt),
        }
        in_maps.append(im)
    return in_maps


def kernel(vectors, Wl, Wr, branching, n_levels, _mode="fp16"):
    _lazy_imports()
    assert int(branching) == B and int(n_levels) == L
    vectors = np.asarray(vectors)
    assert vectors.shape == (int(OFFSETS[L]), D), vectors.shape

    nc = _build_nc(mode=_mode)
    in_maps = _prep_inputs(vectors, Wl, Wr)
    try:
        res = run_bass_kernel_spmd(nc, in_maps, core_ids=list(range(N_CORES)),
                                   trace=False)
    except Exception:
        # transient device hiccups clear on a retry
        res = run_bass_kernel_spmd(nc, in_maps, core_ids=list(range(N_CORES)),
                                   trace=False)
    root = res.results[0]["out"]
    return np.asarray(root, dtype=np.float32).reshape(1, D)
